# revision 24
# baseline (speedup 1.0000x reference)
"""Multi-head attention (RoPE + doc-masked causal) on 8 Trainium2 cores.

Sharding: tensor-parallel over heads. H=16 heads -> 2 heads/core.
Each core computes q/k/v projections for its head group (Wq/Wk/Wv column
slices), attention for its 2 heads, and a partial output projection
(Wo row slice). Host sums the 8 partial outputs.

Layout strategy (per core):
  - qT/kT [hd=128, t] computed directly by projection matmuls (lhsT=W chunk,
    rhs=xT chunk); RoPE applied in this layout on DVE with partition-shifted
    PSUM reads.
  - scoresT [s, t] = k @ qT via matmul(lhsT=kT_block, rhs=qT_chunk); exp on
    ACT (scale=1/sqrt(hd) folded in); softmax denominator via ones-matmul;
    PV as outT[hd, t] = v.T @ pT; normalization by 1/l broadcast across
    partitions with a K=1 matmul; final y = outT_scaled.T @ Wo rows.
  - Matmuls in bf16 (fp32 PSUM accumulation; pT rounding cancels in the
    softmax ratio). The 1/l normalization chain stays fp32/fp32r.
  - doc_ids are sorted -> allowed region of scoresT is block-diagonal AND
    causal. The program is specialized at build time: fully-masked
    128x512 tiles are skipped entirely (matmuls included), fully-allowed
    tiles skip masking, boundary tiles get a host-precomputed bf16 0/1
    mask multiply.
"""
import sys

sys.path.insert(0, "/opt/trn_rl_repo")

import numpy as np

import concourse.bacc as bacc
from concourse import bass_utils, mybir
from concourse.tile import TileContext

dt = mybir.dt

B, T, D, H, HD = 1, 2048, 2048, 16, 128
NCORES = 8
HPC = H // NCORES          # heads per core = 2
J = HPC * HD               # head-group width per core = 256
TCH = 512                  # t-chunk (PSUM bank = 512 fp32)
NTCH = T // TCH            # 4
KC = D // 128              # 16 contraction chunks
NTB = TCH // 128           # t-blocks per chunk = 4
SCALE = 1.0 / np.sqrt(HD)
WPIECE = 2                 # weight DMA split granularity (k-chunks per DMA)


def _plan(doc):
    """Per t-chunk: list of (s_block, mask_idx|None); masks: list of [128,512]."""
    doc = np.asarray(doc).astype(np.int64)
    is_sorted = bool(np.all(doc[1:] >= doc[:-1]))
    plans, masks = [], []
    for ic in range(NTCH):
        t0 = ic * TCH
        tcols = np.arange(t0, t0 + TCH)
        if is_sorted:
            s_lo = int(np.searchsorted(doc, doc[t0], side="left"))
        else:
            s_lo = 0  # scan all blocks; allowed.any() filter keeps correctness
        blocks = []
        for b in range(s_lo // 128, (t0 + TCH - 1) // 128 + 1):
            srows = np.arange(b * 128, b * 128 + 128)
            allowed = (srows[:, None] <= tcols[None, :]) & (
                doc[srows][:, None] == doc[tcols][None, :]
            )
            if not allowed.any():
                continue
            cols = np.flatnonzero(allowed.any(axis=0))
            c0, c1 = int(cols[0]), int(cols[-1]) + 1
            if allowed[:, c0:c1].all():
                blocks.append((b, None, c0, c1))
            else:
                masks.append(allowed.astype(np.float32))
                blocks.append((b, len(masks) - 1, c0, c1))
        plans.append(blocks)
    return plans, masks


def _build(plans, n_masks):
    nc = bacc.Bacc("TRN2", target_bir_lowering=False, debug=False)
    f32, f32r, bf16 = dt.float32, dt.float32r, dt.bfloat16

    xT = nc.dram_tensor("xT", [D, T], bf16, kind="ExternalInput").ap()
    wq = nc.dram_tensor("wq", [128, KC, HPC, 128], bf16, kind="ExternalInput").ap()
    wk = nc.dram_tensor("wk", [128, KC, HPC, 128], bf16, kind="ExternalInput").ap()
    wv = nc.dram_tensor("wv", [128, KC, J], bf16, kind="ExternalInput").ap()
    wo = nc.dram_tensor("wo", [128, HPC, D], bf16, kind="ExternalInput").ap()
    cosT = nc.dram_tensor("cosT", [HD, T], bf16, kind="ExternalInput").ap()
    sinT = nc.dram_tensor("sinT", [HD, T], bf16, kind="ExternalInput").ap()
    onesb_d = nc.dram_tensor("onesb", [128, 1], bf16, kind="ExternalInput").ap()
    onesr_d = nc.dram_tensor("onesr", [1, 128], f32, kind="ExternalInput").ap()
    mk = nc.dram_tensor(
        "masks", [max(1, n_masks), 128, TCH], bf16, kind="ExternalInput"
    ).ap()
    y = nc.dram_tensor("y", [T, D], bf16, kind="ExternalOutput").ap()

    MULT = mybir.AluOpType.mult
    EXP = mybir.ActivationFunctionType.Exp

    wq_r, wk_r, wv_r = wq, wk, wv

    with TileContext(nc) as tc:
        with (
            tc.tile_pool(name="consts", bufs=1) as consts,
            tc.tile_pool(name="xtp", bufs=16) as xtp,
            tc.tile_pool(name="rope", bufs=4) as ropep,
            tc.tile_pool(name="ptp", bufs=10) as ptp,
            tc.tile_pool(name="mkp", bufs=6) as mkp,
            tc.tile_pool(name="outp", bufs=2) as outp,
            tc.tile_pool(name="smallp", bufs=4) as smallp,
            tc.tile_pool(name="yp", bufs=8) as yp,
            tc.tile_pool(name="ps", bufs=1, space="PSUM") as ps,
        ):
            # ---- chunk-0 xT loads interleaved with weight pieces so the
            # q/k projection pipeline starts ASAP ----
            wv_sb = consts.tile([128, KC, J], bf16)
            wq_sb = consts.tile([128, KC, HPC, 128], bf16)
            wk_sb = consts.tile([128, KC, HPC, 128], bf16)
            xts0 = []
            for k0 in range(0, KC, WPIECE):
                ksl = slice(k0, k0 + WPIECE)
                nc.sync.dma_start(out=wq_sb[:, ksl], in_=wq_r[:, ksl])
                nc.gpsimd.dma_start(out=wk_sb[:, ksl], in_=wk_r[:, ksl])
                for k in range(k0, k0 + WPIECE):
                    xt_k = xtp.tile([128, TCH], bf16, tag="xt", name=f"xt_0_{k}")
                    eng = nc.sync if k % 2 == 0 else nc.gpsimd
                    eng.dma_start(out=xt_k, in_=xT[k * 128 : (k + 1) * 128, 0:TCH])
                    xts0.append(xt_k)
            for k0 in range(0, KC, WPIECE):
                ksl = slice(k0, k0 + WPIECE)
                nc.gpsimd.dma_start(out=wv_sb[:, ksl], in_=wv_r[:, ksl])
            cos_sb = consts.tile([HD, T], bf16)
            nc.gpsimd.dma_start(out=cos_sb, in_=cosT)
            sin_sb = consts.tile([HD, T], bf16)
            nc.gpsimd.dma_start(out=sin_sb, in_=sinT)
            ones_bf = consts.tile([128, 1], bf16)
            nc.gpsimd.dma_start(out=ones_bf, in_=onesb_d)
            ones_fr = consts.tile([1, 128], f32r)
            nc.gpsimd.dma_start(out=ones_fr, in_=onesr_d.bitcast(f32r))
            wo_sb = consts.tile([128, HPC, D], bf16)
            nc.gpsimd.dma_start(out=wo_sb, in_=wo)

            # full-kernel persistent tensors
            krope_sb = consts.tile([HD, HPC, T], bf16)
            v_sb = consts.tile([128, KC, J], bf16)

            for ic in range(NTCH):
                t0 = ic * TCH
                tsl = slice(t0, t0 + TCH)

                if ic == 0:
                    xts = xts0
                else:
                    xts = []
                    for k in range(KC):
                        xt_k = xtp.tile([128, TCH], bf16, tag="xt", name=f"xt_{ic}_{k}")
                        nc.sync.dma_start(
                            out=xt_k, in_=xT[k * 128 : (k + 1) * 128, tsl]
                        )
                        xts.append(xt_k)

                # ---- q/k projections + RoPE, per j-block ----
                qrope_sb = ropep.tile([HD, HPC, TCH], bf16, tag="qr", name=f"qr_{ic}")
                for w_sb, dname in ((wq_sb, "q"), (wk_sb, "k")):
                    for jb in range(HPC):
                        qk_ps = ps.tile(
                            [128, TCH], f32, tag="pQK", bufs=2,
                            name=f"qkps_{dname}_{ic}_{jb}",
                        )
                        for k in range(KC):
                            nc.tensor.matmul(
                                qk_ps,
                                w_sb[:, k, jb, :],
                                xts[k],
                                start=(k == 0),
                                stop=(k == KC - 1),
                            )
                        # RoPE: out = u*cos + rot(u)*sin; rot = [-u2, u1]
                        csl = cos_sb[:, tsl]
                        ssl = sin_sb[:, tsl]
                        t1 = ropep.tile([HD, TCH], f32, tag="t1", name=f"t1_{dname}_{ic}_{jb}")
                        nc.vector.scalar_tensor_tensor(
                            out=t1, in0=qk_ps, scalar=1.0, in1=csl,
                            op0=MULT, op1=MULT,
                        )
                        t2 = ropep.tile([HD, TCH], f32, tag="t2", name=f"t2_{dname}_{ic}_{jb}")
                        nc.vector.scalar_tensor_tensor(
                            out=t2[0:64, :], in0=qk_ps[64:128, :], scalar=-1.0,
                            in1=ssl[0:64, :], op0=MULT, op1=MULT,
                        )
                        nc.vector.scalar_tensor_tensor(
                            out=t2[64:128, :], in0=qk_ps[0:64, :], scalar=1.0,
                            in1=ssl[64:128, :], op0=MULT, op1=MULT,
                        )
                        if dname == "q":
                            nc.vector.tensor_add(qrope_sb[:, jb, :], t1, t2)
                        else:
                            nc.vector.tensor_add(krope_sb[:, jb, tsl], t1, t2)

                # ---- v projection: 4 t-blocks of [128, 256] ----
                for tb in range(NTB):
                    v_ps = ps.tile([128, J], f32, tag="pA", bufs=2, name=f"vps_{ic}_{tb}")
                    for k in range(KC):
                        nc.tensor.matmul(
                            v_ps,
                            xts[k][:, tb * 128 : (tb + 1) * 128],
                            wv_sb[:, k, :],
                            start=(k == 0),
                            stop=(k == KC - 1),
                        )
                    nc.vector.tensor_copy(v_sb[:, ic * NTB + tb, :], v_ps)

                # ---- load boundary masks for this chunk ----
                mtiles = {}
                for b, mi, _, _ in plans[ic]:
                    if mi is not None:
                        mt = mkp.tile([128, TCH], bf16, tag="mk", name=f"mk_{ic}_{b}")
                        nc.sync.dma_start(out=mt, in_=mk[mi])
                        mtiles[b] = mt

                # ---- attention per head ----
                outT_sb = outp.tile([HD, HPC, TCH], bf16, tag="outT", name=f"outT_{ic}")
                for h in range(HPC):
                    pts = []
                    for b, mi, c0, c1 in plans[ic]:
                        sc_ps = ps.tile(
                            [128, TCH], f32, tag="pS", bufs=2, name=f"scps_{ic}_{h}_{b}"
                        )
                        nc.tensor.matmul(
                            sc_ps[:, c0:c1],
                            krope_sb[:, h, b * 128 : (b + 1) * 128],
                            qrope_sb[:, h, c0:c1],
                            start=True,
                            stop=True,
                        )
                        pt = ptp.tile([128, TCH], bf16, tag="pt", name=f"pt_{ic}_{h}_{b}")
                        nc.scalar.activation(
                            pt[:, c0:c1], sc_ps[:, c0:c1], EXP, bias=0.0, scale=SCALE
                        )
                        if mi is not None:
                            nc.vector.tensor_tensor(
                                out=pt[:, c0:c1], in0=pt[:, c0:c1],
                                in1=mtiles[b][:, c0:c1], op=MULT,
                            )
                        pts.append((b, pt, c0, c1))

                    # PV + denominator accumulation (per-element has_written
                    # semantics make subrange accumulation correct)
                    o_ps = ps.tile([HD, TCH], f32, tag="pO", name=f"ops_{ic}_{h}")
                    d_ps = ps.tile([1, TCH], f32, tag="pD", name=f"dps_{ic}_{h}")
                    nblk = len(pts)
                    for i, (b, pt, c0, c1) in enumerate(pts):
                        nc.tensor.matmul(
                            o_ps[:, c0:c1],
                            v_sb[:, b, h * HD : (h + 1) * HD],
                            pt[:, c0:c1],
                            start=(i == 0),
                            stop=(i == nblk - 1),
                        )
                        nc.tensor.matmul(
                            d_ps[:, c0:c1],
                            ones_bf,
                            pt[:, c0:c1],
                            start=(i == 0),
                            stop=(i == nblk - 1),
                        )

                    # free the PV bank right away; broadcast l, then take the
                    # reciprocal on the broadcast tile, scale from SBUF
                    o_sb = smallp.tile([HD, TCH], f32, tag="osb", name=f"osb_{ic}_{h}")
                    nc.vector.tensor_copy(o_sb, o_ps)
                    d_r = smallp.tile([1, TCH], f32r, tag="dr", name=f"dr_{ic}_{h}")
                    nc.vector.tensor_copy(d_r, d_ps)
                    bc_ps = ps.tile([128, TCH], f32, tag="pD", name=f"bcps_{ic}_{h}")
                    nc.tensor.matmul(bc_ps, ones_fr, d_r, start=True, stop=True)
                    rec_sb = smallp.tile([128, TCH], f32, tag="rec", name=f"rec_{ic}_{h}")
                    nc.vector.reciprocal_approx_fast(out=rec_sb, in_=bc_ps)
                    nc.vector.scalar_tensor_tensor(
                        out=outT_sb[:, h, :], in0=o_sb, scalar=1.0, in1=rec_sb,
                        op0=MULT, op1=MULT,
                    )

                # ---- output projection: y[t_chunk, :] partial ----
                for tb in range(NTB):
                    trow = t0 + tb * 128
                    for dc in range(D // TCH):
                        y_ps = ps.tile(
                            [128, TCH], f32, tag="pA", bufs=2, name=f"yps_{ic}_{tb}_{dc}"
                        )
                        for h in range(HPC):
                            nc.tensor.matmul(
                                y_ps,
                                outT_sb[:, h, tb * 128 : (tb + 1) * 128],
                                wo_sb[:, h, dc * TCH : (dc + 1) * TCH],
                                start=(h == 0),
                                stop=(h == HPC - 1),
                            )
                        y_sb = yp.tile([128, TCH], bf16, tag="y", name=f"y_{ic}_{tb}_{dc}")
                        if (tb + dc) % 2 == 0:
                            nc.vector.tensor_copy(y_sb, y_ps)
                        else:
                            nc.scalar.copy(y_sb, y_ps)
                        nc.sync.dma_start(
                            out=y[trow : trow + 128, dc * TCH : (dc + 1) * TCH],
                            in_=y_sb,
                        )

    nc.compile()
    return nc


_CACHE = {}
_LAST_RESULTS = None


def _get_program(doc):
    key = doc.tobytes()
    if key not in _CACHE:
        plans, masks = _plan(doc)
        nc = _build(plans, len(masks))
        _CACHE[key] = (nc, masks)
    return _CACHE[key]


def kernel(x, Wq, Wk, Wv, Wo, sin, cos, doc_ids, **kwargs):
    import ml_dtypes

    bf = ml_dtypes.bfloat16
    x = np.asarray(x, dtype=np.float32)
    sin = np.asarray(sin, dtype=np.float32)
    cos = np.asarray(cos, dtype=np.float32)
    doc = np.asarray(doc_ids, dtype=np.int32).reshape(-1)

    nc, masks = _get_program(doc)

    xT = np.ascontiguousarray(x.reshape(T, D).T).astype(bf)
    Wq = np.asarray(Wq, dtype=np.float32).astype(bf)
    Wk = np.asarray(Wk, dtype=np.float32).astype(bf)
    Wv = np.asarray(Wv, dtype=np.float32).astype(bf)
    Wo = np.asarray(Wo, dtype=np.float32).astype(bf)
    cosT = np.ascontiguousarray(cos.T).astype(bf)
    sinT = np.ascontiguousarray(sin.T).astype(bf)
    onesb = np.ones((128, 1), bf)
    onesr = np.ones((1, 128), np.float32)
    mk = (
        np.ascontiguousarray(np.stack(masks)).astype(bf)
        if masks
        else np.zeros((1, 128, TCH), bf)
    )

    in_maps = []
    for c in range(NCORES):
        jsl = slice(c * J, (c + 1) * J)
        wq_c = Wq[:, jsl].reshape(KC, 128, HPC, 128).transpose(1, 0, 2, 3)
        wk_c = Wk[:, jsl].reshape(KC, 128, HPC, 128).transpose(1, 0, 2, 3)
        wv_c = Wv[:, jsl].reshape(KC, 128, J).transpose(1, 0, 2)
        wo_c = Wo[jsl, :].reshape(HPC, 128, D).transpose(1, 0, 2)
        in_maps.append(
            {
                "xT": xT,
                "wq": np.ascontiguousarray(wq_c),
                "wk": np.ascontiguousarray(wk_c),
                "wv": np.ascontiguousarray(wv_c),
                "wo": np.ascontiguousarray(wo_c),
                "cosT": cosT,
                "sinT": sinT,
                "onesb": onesb,
                "onesr": onesr,
                "masks": mk,
            }
        )

    res = bass_utils.run_bass_kernel_spmd(
        nc, in_maps, core_ids=list(range(NCORES)), **kwargs
    )
    global _LAST_RESULTS
    _LAST_RESULTS = res
    y = np.zeros((T, D), np.float64)
    for c in range(NCORES):
        y += res.results[c]["y"].astype(np.float64)
    return y.reshape(B, T, D).astype(np.float32)


# revision 25
# speedup vs baseline: 1.0014x; 1.0014x over previous
"""Multi-head attention (RoPE + doc-masked causal) on 8 Trainium2 cores.

Sharding: tensor-parallel over heads. H=16 heads -> 2 heads/core.
Each core computes q/k/v projections for its head group (Wq/Wk/Wv column
slices), attention for its 2 heads, and a partial output projection
(Wo row slice). Host sums the 8 partial outputs.

Layout strategy (per core):
  - qT/kT [hd=128, t] computed directly by projection matmuls (lhsT=W chunk,
    rhs=xT chunk); RoPE applied in this layout on DVE with partition-shifted
    PSUM reads.
  - scoresT [s, t] = k @ qT via matmul(lhsT=kT_block, rhs=qT_chunk); exp on
    ACT (scale=1/sqrt(hd) folded in); softmax denominator via ones-matmul;
    PV as outT[hd, t] = v.T @ pT; normalization by 1/l broadcast across
    partitions with a K=1 matmul; final y = outT_scaled.T @ Wo rows.
  - Matmuls in bf16 (fp32 PSUM accumulation; pT rounding cancels in the
    softmax ratio). The 1/l normalization chain stays fp32/fp32r.
  - doc_ids are sorted -> allowed region of scoresT is block-diagonal AND
    causal. The program is specialized at build time: fully-masked
    128x512 tiles are skipped entirely (matmuls included), fully-allowed
    tiles skip masking, boundary tiles get a host-precomputed bf16 0/1
    mask multiply.
"""
import sys

sys.path.insert(0, "/opt/trn_rl_repo")

import numpy as np

import concourse.bacc as bacc
from concourse import bass_utils, mybir
from concourse.tile import TileContext

dt = mybir.dt

B, T, D, H, HD = 1, 2048, 2048, 16, 128
NCORES = 8
HPC = H // NCORES          # heads per core = 2
J = HPC * HD               # head-group width per core = 256
TCH = 512                  # t-chunk (PSUM bank = 512 fp32)
NTCH = T // TCH            # 4
KC = D // 128              # 16 contraction chunks
NTB = TCH // 128           # t-blocks per chunk = 4
SCALE = 1.0 / np.sqrt(HD)
WPIECE = 2                 # weight DMA split granularity (k-chunks per DMA)


def _plan(doc):
    """Per t-chunk: list of (s_block, mask_idx|None); masks: list of [128,512]."""
    doc = np.asarray(doc).astype(np.int64)
    is_sorted = bool(np.all(doc[1:] >= doc[:-1]))
    plans, masks = [], []
    for ic in range(NTCH):
        t0 = ic * TCH
        tcols = np.arange(t0, t0 + TCH)
        if is_sorted:
            s_lo = int(np.searchsorted(doc, doc[t0], side="left"))
        else:
            s_lo = 0  # scan all blocks; allowed.any() filter keeps correctness
        blocks = []
        for b in range(s_lo // 128, (t0 + TCH - 1) // 128 + 1):
            srows = np.arange(b * 128, b * 128 + 128)
            allowed = (srows[:, None] <= tcols[None, :]) & (
                doc[srows][:, None] == doc[tcols][None, :]
            )
            if not allowed.any():
                continue
            cols = np.flatnonzero(allowed.any(axis=0))
            c0, c1 = int(cols[0]), int(cols[-1]) + 1
            if allowed[:, c0:c1].all():
                blocks.append((b, None, c0, c1))
            else:
                masks.append(allowed.astype(np.float32))
                blocks.append((b, len(masks) - 1, c0, c1))
        plans.append(blocks)
    return plans, masks


def _build(plans, n_masks):
    nc = bacc.Bacc("TRN2", target_bir_lowering=False, debug=False)
    f32, f32r, bf16 = dt.float32, dt.float32r, dt.bfloat16

    xT = nc.dram_tensor("xT", [D, T], bf16, kind="ExternalInput").ap()
    wq = nc.dram_tensor("wq", [128, KC, HPC, 128], bf16, kind="ExternalInput").ap()
    wk = nc.dram_tensor("wk", [128, KC, HPC, 128], bf16, kind="ExternalInput").ap()
    wv = nc.dram_tensor("wv", [128, KC, J], bf16, kind="ExternalInput").ap()
    wo = nc.dram_tensor("wo", [128, HPC, D], bf16, kind="ExternalInput").ap()
    cosT = nc.dram_tensor("cosT", [HD, T], bf16, kind="ExternalInput").ap()
    sinT = nc.dram_tensor("sinT", [HD, T], bf16, kind="ExternalInput").ap()
    onesb_d = nc.dram_tensor("onesb", [128, 1], bf16, kind="ExternalInput").ap()
    onesr_d = nc.dram_tensor("onesr", [1, 128], f32, kind="ExternalInput").ap()
    mk = nc.dram_tensor(
        "masks", [max(1, n_masks), 128, TCH], bf16, kind="ExternalInput"
    ).ap()
    y = nc.dram_tensor("y", [T, D], bf16, kind="ExternalOutput").ap()

    MULT = mybir.AluOpType.mult
    EXP = mybir.ActivationFunctionType.Exp

    wq_r, wk_r, wv_r = wq, wk, wv

    with TileContext(nc) as tc:
        with (
            tc.tile_pool(name="consts", bufs=1) as consts,
            tc.tile_pool(name="xtp", bufs=16) as xtp,
            tc.tile_pool(name="rope", bufs=4) as ropep,
            tc.tile_pool(name="ptp", bufs=10) as ptp,
            tc.tile_pool(name="outp", bufs=2) as outp,
            tc.tile_pool(name="smallp", bufs=4) as smallp,
            tc.tile_pool(name="yp", bufs=8) as yp,
            tc.tile_pool(name="ps", bufs=1, space="PSUM") as ps,
        ):
            # ---- chunk-0 xT loads interleaved with weight pieces so the
            # q/k projection pipeline starts ASAP ----
            wv_sb = consts.tile([128, KC, J], bf16)
            wq_sb = consts.tile([128, KC, HPC, 128], bf16)
            wk_sb = consts.tile([128, KC, HPC, 128], bf16)
            xts0 = []
            for k0 in range(0, KC, WPIECE):
                ksl = slice(k0, k0 + WPIECE)
                nc.sync.dma_start(out=wq_sb[:, ksl], in_=wq_r[:, ksl])
                nc.gpsimd.dma_start(out=wk_sb[:, ksl], in_=wk_r[:, ksl])
                for k in range(k0, k0 + WPIECE):
                    xt_k = xtp.tile([128, TCH], bf16, tag="xt", name=f"xt_0_{k}")
                    eng = nc.sync if k % 2 == 0 else nc.gpsimd
                    eng.dma_start(out=xt_k, in_=xT[k * 128 : (k + 1) * 128, 0:TCH])
                    xts0.append(xt_k)
            for k0 in range(0, KC, WPIECE):
                ksl = slice(k0, k0 + WPIECE)
                nc.gpsimd.dma_start(out=wv_sb[:, ksl], in_=wv_r[:, ksl])
            cos_sb = consts.tile([HD, T], bf16)
            nc.gpsimd.dma_start(out=cos_sb, in_=cosT)
            sin_sb = consts.tile([HD, T], bf16)
            nc.gpsimd.dma_start(out=sin_sb, in_=sinT)
            ones_bf = consts.tile([128, 1], bf16)
            nc.gpsimd.dma_start(out=ones_bf, in_=onesb_d)
            ones_fr = consts.tile([1, 128], f32r)
            nc.gpsimd.dma_start(out=ones_fr, in_=onesr_d.bitcast(f32r))
            wo_sb = consts.tile([128, HPC, D], bf16)
            nc.gpsimd.dma_start(out=wo_sb, in_=wo)

            # full-kernel persistent tensors
            krope_sb = consts.tile([HD, HPC, T], bf16)
            v_sb = consts.tile([128, KC, J], bf16)

            # all boundary masks preloaded once (tiny)
            mk_tiles = []
            for mi in range(n_masks):
                mt = consts.tile([128, TCH], bf16, name=f"mkt_{mi}")
                nc.gpsimd.dma_start(out=mt, in_=mk[mi])
                mk_tiles.append(mt)

            for ic in range(NTCH):
                t0 = ic * TCH
                tsl = slice(t0, t0 + TCH)

                if ic == 0:
                    xts = xts0
                else:
                    xts = []
                    for k in range(KC):
                        xt_k = xtp.tile([128, TCH], bf16, tag="xt", name=f"xt_{ic}_{k}")
                        eng = nc.sync if k % 2 == 0 else nc.gpsimd
                        eng.dma_start(
                            out=xt_k, in_=xT[k * 128 : (k + 1) * 128, tsl]
                        )
                        xts.append(xt_k)

                # ---- q/k projections + RoPE, per j-block ----
                qrope_sb = ropep.tile([HD, HPC, TCH], bf16, tag="qr", name=f"qr_{ic}")
                for w_sb, dname in ((wq_sb, "q"), (wk_sb, "k")):
                    for jb in range(HPC):
                        qk_ps = ps.tile(
                            [128, TCH], f32, tag="pQK", bufs=2,
                            name=f"qkps_{dname}_{ic}_{jb}",
                        )
                        for k in range(KC):
                            nc.tensor.matmul(
                                qk_ps,
                                w_sb[:, k, jb, :],
                                xts[k],
                                start=(k == 0),
                                stop=(k == KC - 1),
                            )
                        # RoPE: out = u*cos + rot(u)*sin; rot = [-u2, u1]
                        csl = cos_sb[:, tsl]
                        ssl = sin_sb[:, tsl]
                        t1 = ropep.tile([HD, TCH], f32, tag="t1", name=f"t1_{dname}_{ic}_{jb}")
                        nc.vector.scalar_tensor_tensor(
                            out=t1, in0=qk_ps, scalar=1.0, in1=csl,
                            op0=MULT, op1=MULT,
                        )
                        t2 = ropep.tile([HD, TCH], f32, tag="t2", name=f"t2_{dname}_{ic}_{jb}")
                        nc.vector.scalar_tensor_tensor(
                            out=t2[0:64, :], in0=qk_ps[64:128, :], scalar=-1.0,
                            in1=ssl[0:64, :], op0=MULT, op1=MULT,
                        )
                        nc.vector.scalar_tensor_tensor(
                            out=t2[64:128, :], in0=qk_ps[0:64, :], scalar=1.0,
                            in1=ssl[64:128, :], op0=MULT, op1=MULT,
                        )
                        if dname == "q":
                            nc.vector.tensor_add(qrope_sb[:, jb, :], t1, t2)
                        else:
                            nc.vector.tensor_add(krope_sb[:, jb, tsl], t1, t2)

                # ---- v projection: 4 t-blocks of [128, 256] ----
                for tb in range(NTB):
                    v_ps = ps.tile([128, J], f32, tag="pA", bufs=2, name=f"vps_{ic}_{tb}")
                    for k in range(KC):
                        nc.tensor.matmul(
                            v_ps,
                            xts[k][:, tb * 128 : (tb + 1) * 128],
                            wv_sb[:, k, :],
                            start=(k == 0),
                            stop=(k == KC - 1),
                        )
                    nc.vector.tensor_copy(v_sb[:, ic * NTB + tb, :], v_ps)

                mtiles = {b: mk_tiles[mi] for b, mi, _, _ in plans[ic] if mi is not None}

                # ---- attention per head ----
                outT_sb = outp.tile([HD, HPC, TCH], bf16, tag="outT", name=f"outT_{ic}")
                for h in range(HPC):
                    pts = []
                    for b, mi, c0, c1 in plans[ic]:
                        sc_ps = ps.tile(
                            [128, TCH], f32, tag="pS", bufs=2, name=f"scps_{ic}_{h}_{b}"
                        )
                        nc.tensor.matmul(
                            sc_ps[:, c0:c1],
                            krope_sb[:, h, b * 128 : (b + 1) * 128],
                            qrope_sb[:, h, c0:c1],
                            start=True,
                            stop=True,
                        )
                        pt = ptp.tile([128, TCH], bf16, tag="pt", name=f"pt_{ic}_{h}_{b}")
                        nc.scalar.activation(
                            pt[:, c0:c1], sc_ps[:, c0:c1], EXP, bias=0.0, scale=SCALE
                        )
                        if mi is not None:
                            nc.vector.tensor_tensor(
                                out=pt[:, c0:c1], in0=pt[:, c0:c1],
                                in1=mtiles[b][:, c0:c1], op=MULT,
                            )
                        pts.append((b, pt, c0, c1))

                    # PV + denominator accumulation (per-element has_written
                    # semantics make subrange accumulation correct)
                    o_ps = ps.tile([HD, TCH], f32, tag="pO", name=f"ops_{ic}_{h}")
                    d_ps = ps.tile([1, TCH], f32, tag="pD", name=f"dps_{ic}_{h}")
                    nblk = len(pts)
                    for i, (b, pt, c0, c1) in enumerate(pts):
                        nc.tensor.matmul(
                            o_ps[:, c0:c1],
                            v_sb[:, b, h * HD : (h + 1) * HD],
                            pt[:, c0:c1],
                            start=(i == 0),
                            stop=(i == nblk - 1),
                        )
                        nc.tensor.matmul(
                            d_ps[:, c0:c1],
                            ones_bf,
                            pt[:, c0:c1],
                            start=(i == 0),
                            stop=(i == nblk - 1),
                        )

                    # free the PV bank right away; broadcast l, then take the
                    # reciprocal on the broadcast tile, scale from SBUF
                    o_sb = smallp.tile([HD, TCH], f32, tag="osb", name=f"osb_{ic}_{h}")
                    nc.vector.tensor_copy(o_sb, o_ps)
                    d_r = smallp.tile([1, TCH], f32r, tag="dr", name=f"dr_{ic}_{h}")
                    nc.vector.tensor_copy(d_r, d_ps)
                    bc_ps = ps.tile([128, TCH], f32, tag="pD", name=f"bcps_{ic}_{h}")
                    nc.tensor.matmul(bc_ps, ones_fr, d_r, start=True, stop=True)
                    rec_sb = smallp.tile([128, TCH], f32, tag="rec", name=f"rec_{ic}_{h}")
                    nc.vector.reciprocal_approx_fast(out=rec_sb, in_=bc_ps)
                    nc.vector.scalar_tensor_tensor(
                        out=outT_sb[:, h, :], in0=o_sb, scalar=1.0, in1=rec_sb,
                        op0=MULT, op1=MULT,
                    )

                # ---- output projection: y[t_chunk, :] partial ----
                for tb in range(NTB):
                    trow = t0 + tb * 128
                    for dc in range(D // TCH):
                        y_ps = ps.tile(
                            [128, TCH], f32, tag="pA", bufs=2, name=f"yps_{ic}_{tb}_{dc}"
                        )
                        for h in range(HPC):
                            nc.tensor.matmul(
                                y_ps,
                                outT_sb[:, h, tb * 128 : (tb + 1) * 128],
                                wo_sb[:, h, dc * TCH : (dc + 1) * TCH],
                                start=(h == 0),
                                stop=(h == HPC - 1),
                            )
                        y_sb = yp.tile([128, TCH], bf16, tag="y", name=f"y_{ic}_{tb}_{dc}")
                        if (tb + dc) % 2 == 0:
                            nc.vector.tensor_copy(y_sb, y_ps)
                        else:
                            nc.scalar.copy(y_sb, y_ps)
                        nc.sync.dma_start(
                            out=y[trow : trow + 128, dc * TCH : (dc + 1) * TCH],
                            in_=y_sb,
                        )

    nc.compile()
    return nc


_CACHE = {}
_LAST_RESULTS = None


def _get_program(doc):
    key = doc.tobytes()
    if key not in _CACHE:
        plans, masks = _plan(doc)
        nc = _build(plans, len(masks))
        _CACHE[key] = (nc, masks)
    return _CACHE[key]


def kernel(x, Wq, Wk, Wv, Wo, sin, cos, doc_ids, **kwargs):
    import ml_dtypes

    bf = ml_dtypes.bfloat16
    x = np.asarray(x, dtype=np.float32)
    sin = np.asarray(sin, dtype=np.float32)
    cos = np.asarray(cos, dtype=np.float32)
    doc = np.asarray(doc_ids, dtype=np.int32).reshape(-1)

    nc, masks = _get_program(doc)

    xT = np.ascontiguousarray(x.reshape(T, D).T).astype(bf)
    Wq = np.asarray(Wq, dtype=np.float32).astype(bf)
    Wk = np.asarray(Wk, dtype=np.float32).astype(bf)
    Wv = np.asarray(Wv, dtype=np.float32).astype(bf)
    Wo = np.asarray(Wo, dtype=np.float32).astype(bf)
    cosT = np.ascontiguousarray(cos.T).astype(bf)
    sinT = np.ascontiguousarray(sin.T).astype(bf)
    onesb = np.ones((128, 1), bf)
    onesr = np.ones((1, 128), np.float32)
    mk = (
        np.ascontiguousarray(np.stack(masks)).astype(bf)
        if masks
        else np.zeros((1, 128, TCH), bf)
    )

    in_maps = []
    for c in range(NCORES):
        jsl = slice(c * J, (c + 1) * J)
        wq_c = Wq[:, jsl].reshape(KC, 128, HPC, 128).transpose(1, 0, 2, 3)
        wk_c = Wk[:, jsl].reshape(KC, 128, HPC, 128).transpose(1, 0, 2, 3)
        wv_c = Wv[:, jsl].reshape(KC, 128, J).transpose(1, 0, 2)
        wo_c = Wo[jsl, :].reshape(HPC, 128, D).transpose(1, 0, 2)
        in_maps.append(
            {
                "xT": xT,
                "wq": np.ascontiguousarray(wq_c),
                "wk": np.ascontiguousarray(wk_c),
                "wv": np.ascontiguousarray(wv_c),
                "wo": np.ascontiguousarray(wo_c),
                "cosT": cosT,
                "sinT": sinT,
                "onesb": onesb,
                "onesr": onesr,
                "masks": mk,
            }
        )

    res = bass_utils.run_bass_kernel_spmd(
        nc, in_maps, core_ids=list(range(NCORES)), **kwargs
    )
    global _LAST_RESULTS
    _LAST_RESULTS = res
    y = np.zeros((T, D), np.float64)
    for c in range(NCORES):
        y += res.results[c]["y"].astype(np.float64)
    return y.reshape(B, T, D).astype(np.float32)
